# revision 21
# baseline (speedup 1.0000x reference)
"""Trainium2 Bass kernel for the CerealBar VIN problem — offset-coords layout.

Self-contained: hardcodes shapes B=512, E=25, 6 orientations, gamma=0.9,
8-core batch sharding (64 samples/core).

Math: the VIN update per orientation o is
    v'[o] = max(sh_{+d(o)} v[o], sh_{-d(o)} v[o], v[o+1], v[o-1])
    Y'    = v' + G_k,   G_k = gmB * gamma^-(k+1)   (rescaled domain,
            gmB = goals with -100 at obstacles/out-of-grid)
Iteration happens in ORIGINAL OFFSET coordinates (25x25, every cell valid)
instead of the reference's 37x25 axial embedding: hex-neighbor shifts become
column-parity-dependent storage offsets.  Even/odd columns are covered by one
instruction via a [2C + (delta_odd - delta_even)]-stride inner column-pair AP
dim (the parity delta difference is +1 for every dv!=0 direction).  This cuts
DVE work per pass from 988 to ~700 cols/plane vs the axial layout.

Device layout: partition p = h*64 + i -> sample i, column-half h.
Per orientation slot: 15 col-positions x 27 rows, column-major (SLOT=405).
half0 stores v=-1..13 at jj=0..14 (jj=0 dead BIG, jj=14 = halo col v=13);
half1 stores v=11..25 (jj=0 = halo col v=11, jj=14 dead).  Column v=12 is
computed redundantly by both halves so storage-parity == v-parity in both.
Rows ss=0/26 are BIG borders.  All constants are host-baked into the initial
W load; no device memsets.

G tensors are generated ON DEVICE each iteration by the (otherwise idle)
Activation engine: g[(k+1)%2] = Copy(gmaster) * gamma^-(k+2) — no per-
iteration HBM streaming, no DMA-queue congestion at startup.

Halo: one column per direction per iteration.  Z for the owner columns is
computed early into a staging tile, DMA'd cross-half into the peer's halo
column of wb (sync + scalar queues), and the +G add for the halo column runs
at the START of the next iteration (after the dv=0 passes), so the DMA has a
multi-us window and the DVE never stalls on it.
"""
import json
import sys

sys.path.insert(0, "/opt/trn_rl_repo")

import numpy as np

import concourse.bass as bass
import concourse.mybir as mybir
from concourse.ap import AP
from concourse.bass_utils import run_bass_kernel_spmd
from concourse.tile import TileContext

E = 25
GAMMA = 0.9
BIG = -100.0
C = 27            # rows per column (u=-1..25)
NCOL = 15         # col-positions per slot (jj=0..14)
S = C * NCOL      # 405 elems per slot
PLANE = 6 * S     # 2430 per partition
PAD = 32          # read-overrun pad after the 6 slots
N_CORES = 8
BPC = 64

# axial-basis hex directions per orientation
D_AX = [(0, 1), (1, 0), (1, -1), (0, -1), (-1, 0), (-1, 1)]

import os as _os

if _os.environ.get("KDT", "fp16") == "fp16":
    DTYPE = mybir.dt.float16
    NP_DT = np.float16
else:
    DTYPE = mybir.dt.float32
    NP_DT = np.float32

TRACE = False
LAST_RESULT = None


def _delta(o, par):
    """Storage delta (elements) for shifting by hex dir o at columns of
    v-parity par (0=even).  ds = dr + ((v+dv)//2 - v//2), delta = ds + dv*C."""
    dr, dv = D_AX[o]
    du = dr + ((par + dv) // 2 - par // 2)
    return du + dv * C


# ---------------------------------------------------------------- BIR fixups
def _split_multi_waits(bir):
    """The installed walrus rejects >1 sync wait per instruction; hoist
    extras onto single-wait NoOps inserted before it on the same engine."""
    for fn in bir.get("functions", []):
        for blk in fn.get("blocks", []):
            out = []
            for ins in blk.get("instructions", []):
                si = ins.get("sync_info")
                waits = (si or {}).get("on_wait") or []
                if len(waits) > 1:
                    for k, w in enumerate(waits[:-1]):
                        out.append({
                            "debug": ins.get("debug", 0),
                            "engine": ins["engine"],
                            "ins": [], "outs": [],
                            "name": f"{ins['name']}_w{k}",
                            "opcode": "NoOp",
                            "sync_info": {"on_wait": [w], "on_update": []},
                            "text_hint": "split_wait",
                        })
                    si["on_wait"] = [waits[-1]]
                out.append(ins)
            blk["instructions"] = out
    return bir


def _install_compat(nc):
    orig = nc.to_json_bytes

    def patched():
        return json.dumps(_split_multi_waits(json.loads(orig()))).encode()

    nc.to_json_bytes = patched


# ---------------------------------------------------------------- kernel build
def _rap(t, off, pairs):
    """Raw AP over pool tile t (full 128 partitions) with free dims pairs."""
    return AP(t.tensor, int(t.offset) + off,
              [list(t.ap[0])] + [list(p) for p in pairs])


def _raph(t, half, off, pairs):
    """Raw AP over one 64-partition half of pool tile t."""
    base = t[64:128] if half else t[0:64]
    return AP(t.tensor, int(base.offset) + off,
              [list(base.ap[0])] + [list(p) for p in pairs])


def build_nc(n_iter):
    nc = bass.Bass()
    _install_compat(nc)
    mx = mybir.AluOpType.max
    add = mybir.AluOpType.add
    copy_f = mybir.ActivationFunctionType.Copy

    wi_d = nc.declare_dram_parameter("winit", [128, PLANE + PAD], DTYPE,
                                     isOutput=False)
    w_d = nc.declare_dram_parameter("w", [128, PLANE], DTYPE, isOutput=True)

    with TileContext(nc) as tc:
        with tc.tile_pool(name="p", bufs=1) as pool:
            wb = pool.tile([128, PLANE + PAD], DTYPE)
            t0 = pool.tile([128, PLANE], DTYPE)   # X, then Z
            t1 = pool.tile([128, PLANE], DTYPE)   # M2
            gm = pool.tile([128, PLANE], DTYPE)   # G_0 = gmB * gamma^-1 master
            g_even = pool.tile([128, PLANE], DTYPE)   # G ping-pong (k >= 1)
            g_odd = pool.tile([128, PLANE], DTYPE)
            gg = [g_even, g_odd]
            hr = pool.tile([128, 160], DTYPE)   # halo receive (cross-half Z)

            q = PLANE // 4
            nc.sync.dma_start(out=wb[:, 0:q], in_=wi_d[:, 0:q])
            nc.scalar.dma_start(out=wb[:, q:2 * q], in_=wi_d[:, q:2 * q])
            nc.sync.dma_start(out=wb[:, 2 * q:3 * q], in_=wi_d[:, 2 * q:3 * q])
            nc.scalar.dma_start(out=wb[:, 3 * q:PLANE + PAD],
                                in_=wi_d[:, 3 * q:PLANE + PAD])
            # G_0 master = wb * gamma^-1 (Act engine, before addI(0) clobbers
            # wb); G_k for k>=1 is gm * gamma^-k, generated one iter ahead.
            nc.scalar.activation(out=gm[:, :], in_=_rap(wb, 0, [[1, PLANE]]),
                                 func=copy_f, bias=0.0,
                                 scale=float(GAMMA ** -1))

            def x_pair14():
                """dv=0 pair (1,4): parity-free, all 13 data cols."""
                base = S + C + 1
                out = _rap(t0, base, [[3 * S, 2], [C, 13], [1, 25]])
                in0 = _rap(wb, base + 1, [[3 * S - 2, 2], [C, 13], [1, 25]])
                in1 = _rap(wb, base - 1, [[3 * S + 2, 2], [C, 13], [1, 25]])
                nc.vector.tensor_tensor(out=out, in0=in0, in1=in1, op=mx)

            def x_pair(a, b, par):
                """t0[{a,b}] = max(sh_{+d} wb, sh_{-d} wb) on par-parity cols
                (the DVE ISA allows only 3 free AP dims, so even/odd columns
                are separate instructions)."""
                jj0, ncols = (1, 7) if par == 0 else (2, 6)
                da, db = _delta(a, par), _delta(b, par)
                base = a * S + jj0 * C + 1
                step = (b - a) * S
                out = _rap(t0, base, [[step, 2], [2 * C, ncols], [1, 25]])
                in0 = _rap(wb, base + da,
                           [[step + (db - da), 2], [2 * C, ncols], [1, 25]])
                in1 = _rap(wb, base + db,
                           [[step + (da - db), 2], [2 * C, ncols], [1, 25]])
                nc.vector.tensor_tensor(out=out, in0=in0, in1=in1, op=mx)

            dcols = [[C, 13], [1, 25]]   # data cols jj=1..13, rows 1..25

            def addH(git):
                """halo cols of wb <- DMA-landed Z (in hr) + G_git.  Writing
                wb from the DVE (not the DMA) avoids a false WAW between the
                halo DMA and addI's disjoint-column writes to wb."""
                g = gm if git == 0 else gg[git % 2]
                nc.vector.tensor_tensor(
                    out=_raph(wb, 0, 14 * C + 1, [[S, 6], [1, 25]]),
                    in0=_raph(hr, 0, 0, [[25, 6], [1, 25]]),
                    in1=_raph(g, 0, 14 * C + 1, [[S, 6], [1, 25]]), op=add)
                nc.vector.tensor_tensor(
                    out=_raph(wb, 1, 1, [[S, 6], [1, 25]]),
                    in0=_raph(hr, 1, 0, [[25, 6], [1, 25]]),
                    in1=_raph(g, 1, 1, [[S, 6], [1, 25]]), op=add)

            for it in range(n_iter):
                g = gm if it == 0 else gg[it % 2]
                # ---- passes that need no halo columns
                x_pair14()
                nc.vector.tensor_tensor(          # M2 mid: t1[1:5]
                    out=_rap(t1, S + C + 1, [[S, 4]] + dcols),
                    in0=_rap(wb, 2 * S + C + 1, [[S, 4]] + dcols),
                    in1=_rap(wb, C + 1, [[S, 4]] + dcols), op=mx)
                nc.vector.tensor_tensor(          # M2 wrap: t1[0], t1[5]
                    out=_rap(t1, C + 1, [[5 * S, 2]] + dcols),
                    in0=_rap(wb, S + C + 1, [[-S, 2]] + dcols),
                    in1=_rap(wb, 5 * S + C + 1, [[-S, 2]] + dcols), op=mx)
                # ---- halo-col +G for previous iteration (DMA landed long ago)
                if it > 0:
                    addH(it - 1)
                # ---- parity-split shift pairs (read halo cols)
                x_pair(0, 3, 0)
                x_pair(0, 3, 1)
                x_pair(2, 5, 0)
                x_pair(2, 5, 1)
                # ---- Z over all data cols (in place in t0)
                nc.vector.tensor_tensor(
                    out=_rap(t0, C + 1, [[S, 6]] + dcols),
                    in0=_rap(t0, C + 1, [[S, 6]] + dcols),
                    in1=_rap(t1, C + 1, [[S, 6]] + dcols), op=mx)
                # ---- cross-half halo DMA straight out of Z: half1's Z col
                # jj=2 (v=13) -> half0's receive; half0's jj=12 (v=11) ->
                # half1's.  The next iteration's first t0 WRITER (xp14) is
                # ~3us after issue, past the DMA read, so no WAR stall.
                if it < n_iter - 1:
                    nc.sync.dma_start(
                        out=_raph(hr, 0, 0, [[1, 150]]),
                        in_=_raph(t0, 1, 2 * C + 1, [[S, 6], [1, 25]]))
                    nc.scalar.dma_start(
                        out=_raph(hr, 1, 0, [[1, 150]]),
                        in_=_raph(t0, 0, 12 * C + 1, [[S, 6], [1, 25]]))
                # ---- generate G_{it+1} = gm * gamma^-(it+1) on the Act
                # engine.  Emitted after the halo DMA issues so the scalar
                # queue's DMA never queues behind the 2.3us COPY; emitted
                # after addH so the WAR on the overwritten buffer is ordered.
                if it + 1 < n_iter:
                    nc.scalar.activation(
                        out=gg[(it + 1) % 2][:, :], in_=gm[:, :], func=copy_f,
                        bias=0.0, scale=float(GAMMA ** -(it + 1)))
                # ---- Y' = Z + G_it
                if it < n_iter - 1:
                    nc.vector.tensor_tensor(
                        out=_rap(wb, C + 1, [[S, 6]] + dcols),
                        in0=_rap(t0, C + 1, [[S, 6]] + dcols),
                        in1=_rap(g, C + 1, [[S, 6]] + dcols), op=add)
                else:
                    # final iteration: add + ship in shrinking chunks so the
                    # last ship (the tail) is small and overlapped
                    for ci, (s0, ns) in enumerate(((0, 2), (2, 2), (4, 1),
                                                   (5, 1))):
                        nc.vector.tensor_tensor(
                            out=_rap(wb, s0 * S + C + 1, [[S, ns]] + dcols),
                            in0=_rap(t0, s0 * S + C + 1, [[S, ns]] + dcols),
                            in1=_rap(g, s0 * S + C + 1, [[S, ns]] + dcols),
                            op=add)
                        eng = nc.sync if ci % 2 == 0 else nc.scalar
                        eng.dma_start(out=w_d[:, s0 * S:(s0 + ns) * S],
                                      in_=_rap(wb, s0 * S, [[1, ns * S]]))
    return nc


_NC_CACHE = {}


def _get_nc(n_iter):
    if n_iter not in _NC_CACHE:
        _NC_CACHE[n_iter] = build_nc(n_iter)
    return _NC_CACHE[n_iter]


# ---------------------------------------------------------------- host side
def kernel(offset_input_goals, offset_current_state, offset_obstacles,
           num_iterations):
    global LAST_RESULT
    goals = np.asarray(offset_input_goals, np.float32)
    state = np.asarray(offset_current_state)
    obst = np.asarray(offset_obstacles, np.float32)
    n_iter = int(num_iterations)
    B = goals.shape[0]
    assert B == N_CORES * BPC and n_iter >= 1

    m = 1.0 - obst                                          # [B,25,25] free
    gmb = np.where(m[:, None] > 0.5, goals, BIG)            # [B,6,25,25]
    P = np.full((B, 6, E + 2, E + 2), BIG, np.float32)      # [u+1, v+1]
    P[:, :, 1:26, 1:26] = gmb

    # column-major halves: [B, 6, 15(jj), 27(ss)] -> [B, 6*405]
    H0 = P[:, :, :, 0:15].swapaxes(2, 3).reshape(B, PLANE)
    H1 = P[:, :, :, 12:27].swapaxes(2, 3).reshape(B, PLANE)

    in_maps = []
    for c in range(N_CORES):
        s = slice(c * BPC, (c + 1) * BPC)
        gmc = np.concatenate([H0[s], H1[s]], 0).astype(NP_DT)   # [128, PLANE]
        wi = np.zeros((128, PLANE + PAD), NP_DT)
        wi[:, :PLANE] = gmc
        in_maps.append({"winit": wi})

    nc = _get_nc(n_iter)
    res = run_bass_kernel_spmd(nc, in_maps, core_ids=list(range(N_CORES)),
                               trace=TRACE)
    LAST_RESULT = res

    w_all = np.stack([np.asarray(res.results[c]["w"], np.float32)
                      for c in range(N_CORES)], 0)          # [8,128,PLANE]
    w_all = w_all.reshape(8, 2, BPC, 6, NCOL, C)
    wh0 = w_all[:, 0].reshape(B, 6, NCOL, C)
    wh1 = w_all[:, 1].reshape(B, 6, NCOL, C)
    W = np.empty((B, 6, E, E), np.float32)                  # [u, v]
    W[:, :, :, 0:13] = wh0[:, :, 1:14, 1:26].transpose(0, 1, 3, 2)
    W[:, :, :, 13:25] = wh1[:, :, 2:14, 1:26].transpose(0, 1, 3, 2)

    alpha = state[:, 0].astype(np.int64)
    u = state[:, 1].astype(np.int64)
    v = state[:, 2].astype(np.int64)
    rot = (alpha + 1) % 6
    bs = np.arange(B)
    w_scale = np.float32(GAMMA ** n_iter)

    def read_w(slot, uu, vv):
        valid = (uu >= 0) & (uu < E) & (vv >= 0) & (vv < E)
        uc = np.clip(uu, 0, E - 1)
        vc = np.clip(vv, 0, E - 1)
        val = W[bs, slot, uc, vc] * w_scale
        return np.where(valid, np.maximum(val, 0.0), 0.0)

    dr = np.array([d[0] for d in D_AX])[rot]
    dv = np.array([d[1] for d in D_AX])[rot]
    ds_f = dr + ((v + dv) // 2 - v // 2)
    ds_b = -dr + ((v - dv) // 2 - v // 2)
    m_pt = m[bs, u, v]

    out = np.zeros((B, 4), np.float32)
    out[:, 0] = m_pt * read_w(rot, u + ds_f, v + dv)
    out[:, 1] = m_pt * read_w(rot, u + ds_b, v - dv)
    out[:, 2] = read_w((rot + 1) % 6, u, v)
    out[:, 3] = read_w((rot + 5) % 6, u, v)
    return out
